# revision 9
# baseline (speedup 1.0000x reference)
"""Trainium2 Bass kernel for CNN+Mamba classifier.

Contract: kernel(**inputs) takes FULL unsharded inputs (numpy), returns FULL
(8, 10) float32 output. Internally shards data-parallel over batch across 8
NeuronCores (1 example per core), with all parameters replicated.

v3 architecture:
  - Embedding gather via gpsimd indirect DMA + PE transposes (proven path).
  - Selective scan tiled channel-major: one [128, 2048] tile per state n
    (both 128-channel halves merged along the free dim; dA boundary column
    kept 0 so the recurrence resets between halves). No replication matmuls,
    no PSUM->SBUF copies.
  - B/C rows replicated across partitions by DMA; read twice via 0-stride AP.
  - All scan-phase elementwise work on DVE (GpSimd shares SBUF ports with
    DVE, so offloading there is a wash); exps on Act; y accumulated over
    states via identity-lhsT PSUM matmuls; mean folded into stt accum_out.

Self-contained: hardcodes all shapes; no sibling imports.
"""

import os
from contextlib import ExitStack

import numpy as np
import ml_dtypes

import concourse.bass as bass
import concourse.bacc as bacc
import concourse.tile as tile
from concourse import mybir
from concourse.bass_utils import run_bass_kernel_spmd

FP = mybir.dt.float32
BF = mybir.dt.bfloat16
I32 = mybir.dt.int32

VOCAB, EMB, NCLS, SEQ = 50000, 256, 10, 2048
DM, DI, DS, DCONV, DTR = 128, 256, 16, 4, 8
L = SEQ // 2  # 1024 after maxpool


def _rep2(t_ap, col0, n):
    """in1 AP reading cols [col0, col0+n) of a [128, *] tile twice (0-stride)."""
    full = t_ap[:]
    return bass.AP(tensor=full.tensor, offset=full.offset + col0,
                   ap=[[full.ap[0][0], 128], [0, 2], [1, n]])


def _row_bcast(t_ap, row, n):
    """DMA source AP: row `row` of tile, broadcast to 128 partitions."""
    full = t_ap[:]
    pstep = full.ap[0][0]
    return bass.AP(tensor=full.tensor, offset=full.offset + row * pstep,
                   ap=[[pstep, 1], [0, 128], [1, n]])


def build_program():
    nc = bacc.Bacc("TRN2", target_bir_lowering=False, debug=False, num_devices=8)

    d_ids = nc.dram_tensor("ids", [128, 16], I32, kind="ExternalInput")
    d_emb = nc.dram_tensor("emb", [VOCAB, EMB], BF, kind="ExternalInput")
    d_c1w = nc.dram_tensor("c1w", [128, 5 * 2 * 128], BF, kind="ExternalInput")
    d_xcw = nc.dram_tensor("xcw", [128, 4 * 2 * 128], BF, kind="ExternalInput")
    d_zw = nc.dram_tensor("zw", [128, 2 * 128], BF, kind="ExternalInput")
    d_xpw = nc.dram_tensor("xpw", [128, 2 * 40], BF, kind="ExternalInput")
    d_dtw = nc.dram_tensor("dtw", [8, 2 * 128], BF, kind="ExternalInput")
    d_asc = nc.dram_tensor("asc", [128, 2 * DS], FP, kind="ExternalInput")
    d_opw = nc.dram_tensor("opw", [128, 2 * 128], FP, kind="ExternalInput")
    d_fcw = nc.dram_tensor("fcw", [128, NCLS], FP, kind="ExternalInput")
    d_ident = nc.dram_tensor("ident", [128, 128], BF, kind="ExternalInput")
    d_c1b = nc.dram_tensor("c1b", [128, 1], FP, kind="ExternalInput")
    d_cdb = nc.dram_tensor("cdb", [128, 2], FP, kind="ExternalInput")
    d_dtb = nc.dram_tensor("dtb", [128, 2], FP, kind="ExternalInput")
    d_dvec = nc.dram_tensor("dvec", [128, 2], FP, kind="ExternalInput")
    d_fcb = nc.dram_tensor("fcb", [10, 1], FP, kind="ExternalInput")

    import uuid
    nonce = uuid.uuid4().hex[:12]
    d_nonce = nc.dram_tensor(f"nonce_{nonce}", [1, 1], FP, kind="ExternalInput")
    d_out = nc.dram_tensor("out", [NCLS], FP, kind="ExternalOutput")

    Alu = mybir.AluOpType
    Act = mybir.ActivationFunctionType

    with ExitStack() as ctx:
        tc = ctx.enter_context(tile.TileContext(nc))
        W = ctx.enter_context(tc.tile_pool(name="w", bufs=1))
        nonce_sb = W.tile([1, 1], FP, name="nonce_sb")
        nc.sync.dma_start(out=nonce_sb[:], in_=d_nonce[:])

        def load(dram, shape, dtype=FP):
            t = W.tile(list(shape), dtype, name=f"w_{dram.name}")
            nc.sync.dma_start(out=t[:], in_=dram[:])
            return t

        ids_sb = W.tile([128, 16], I32, name="ids_sb")
        nc.sync.dma_start(out=ids_sb[:], in_=d_ids[:])
        c1w = load(d_c1w, (128, 5 * 2 * 128), BF)
        xcw = load(d_xcw, (128, 4 * 2 * 128), BF)
        zw = load(d_zw, (128, 2 * 128), BF)
        xpw = load(d_xpw, (128, 2 * 40), BF)
        dtw = load(d_dtw, (8, 2 * 128), BF)
        asc = load(d_asc, (128, 2 * DS))
        opw = load(d_opw, (128, 2 * 128))
        fcw = load(d_fcw, (128, NCLS))
        ident = load(d_ident, (128, 128), BF)
        c1b = load(d_c1b, (128, 1))
        cdb = load(d_cdb, (128, 2))
        dtb = load(d_dtb, (128, 2))
        dvec = load(d_dvec, (128, 2))
        fcb = load(d_fcb, (10, 1))

        # ---- persistent intermediates ----
        x_emb = [W.tile([128, SEQ + 4], BF, name=f"x_emb{_}") for _ in range(2)]
        for h in range(2):
            nc.vector.memset(x_emb[h][:, 0:2], 0.0)
            nc.vector.memset(x_emb[h][:, SEQ + 2:SEQ + 4], 0.0)
        relu_sb = W.tile([128, SEQ], BF, name="relu_sb")
        x_pool = W.tile([128, L + 3], BF, name="x_pool")
        nc.vector.memset(x_pool[:, 0:3], 0.0)
        xs_big = W.tile([128, 2 * L], BF, name="xs_big")
        sz_big = W.tile([128, 2 * L], BF, name="sz_big")
        dt_big = W.tile([128, 2 * L], FP, name="dt_big")
        u_big = W.tile([128, 2 * L], BF, name="u_big")
        xdbl_sb = W.tile([40, L], BF, name="xdbl_sb")
        brep = W.tile([128, DS * L], BF, name="brep")
        crep = W.tile([128, DS * L], BF, name="crep")

        # scan working rings; dA col L is the recurrence reset (stays 0)
        NB = 4
        dA_r = [W.tile([128, 2 * L], BF, name=f"dA{i}") for i in range(NB)]
        for i in range(NB):
            nc.vector.memset(dA_r[i][:, L:L + 1], 0.0)
        dBu_r = [W.tile([128, 2 * L], BF, name=f"dBu{i}") for i in range(NB)]
        ht_r = [W.tile([128, 2 * L], BF, name=f"ht{i}") for i in range(NB)]
        hC_r = [W.tile([128, 2 * L], BF, name=f"hC{i}") for i in range(NB)]

        # preload the silu ACT table (first real Act op is silu)
        scratch = W.tile([128, 2], FP, name="act_scratch")
        nc.vector.memset(scratch[:], 1.0)
        nc.scalar.activation(out=scratch[:, 0:1], in_=scratch[:, 0:1],
                             func=Act.Silu, scale=1.0)

        # ============ PHASE 1+2: gather + transpose | conv + relu + max =====
        with tc.tile_pool(name="g", bufs=8) as gp, \
             tc.tile_pool(name="gt", bufs=2, space="PSUM") as gtp, \
             tc.tile_pool(name="wrm", bufs=1, space="PSUM") as wrm, \
             tc.tile_pool(name="cps", bufs=2, space="PSUM") as cpp, \
             tc.tile_pool(name="fe", bufs=3, space="PSUM") as fe:

            def conv_group(g):
                o = 512 * g
                cps = cpp.tile([128, 512], FP, tag="cps")
                for k in range(5):
                    for kh in range(2):
                        nc.tensor.matmul(
                            out=cps[:],
                            lhsT=c1w[:, (k * 2 + kh) * 128:(k * 2 + kh + 1) * 128],
                            rhs=x_emb[kh][:, o + k: o + k + 512],
                            start=(k == 0 and kh == 0), stop=(k == 4 and kh == 1))
                nc.vector.tensor_scalar(
                    out=relu_sb[:, o:o + 512], in0=cps[:],
                    scalar1=c1b[:, 0:1], scalar2=0.0, op0=Alu.add, op1=Alu.max)
                rf = relu_sb[:]
                ev = bass.AP(tensor=rf.tensor, offset=rf.offset + o,
                             ap=[[rf.ap[0][0], 128], [2, 256]])
                od = bass.AP(tensor=rf.tensor, offset=rf.offset + o + 1,
                             ap=[[rf.ap[0][0], 128], [2, 256]])
                nc.vector.tensor_max(out=x_pool[:, 3 + o // 2: 3 + o // 2 + 256],
                                     in0=ev, in1=od)

            wps = wrm.tile([128, 512], FP, name="warm_ps")
            for c in range(16):
                xg = gp.tile([128, EMB], BF, tag="xg")
                nc.gpsimd.indirect_dma_start(
                    out=xg[:], out_offset=None, in_=d_emb[:],
                    in_offset=bass.IndirectOffsetOnAxis(ap=ids_sb[:, c:c + 1],
                                                        axis=0))
                for h in range(2):
                    pt = gtp.tile([128, 128], BF, tag="pt")
                    nc.tensor.transpose(out=pt[:], in_=xg[:, 128 * h:128 * (h + 1)],
                                        identity=ident[:])
                    nc.vector.tensor_copy(
                        out=x_emb[h][:, 2 + 128 * c:2 + 128 * (c + 1)], in_=pt[:])
                if 2 <= c < 10:
                    # PE pstate warmup while the gather streams in
                    nc.tensor.matmul(out=wps[:], lhsT=c1w[:, 0:128],
                                     rhs=c1w[:, 0:512], start=True, stop=True)
                # issue conv group g once chunks 0..4g+4 are transposed
                if c >= 4 and c % 4 == 0:
                    conv_group(c // 4 - 1)
            conv_group(3)

            # ====== PHASE 3: in_proj (+ folded depthwise conv) + silu =======
            for o in (0, 512):
                for h in range(2):
                    xcp = fe.tile([128, 512], FP, tag="ip")
                    for k in range(4):
                        nc.tensor.matmul(
                            out=xcp[:],
                            lhsT=xcw[:, (k * 2 + h) * 128:(k * 2 + h + 1) * 128],
                            rhs=x_pool[:, o + k: o + k + 512],
                            start=(k == 0), stop=(k == 3))
                    zp = fe.tile([128, 512], FP, tag="ip")
                    nc.tensor.matmul(
                        out=zp[:], lhsT=zw[:, h * 128:(h + 1) * 128],
                        rhs=x_pool[:, 3 + o: 3 + o + 512], start=True, stop=True)
                    nc.scalar.activation(out=xs_big[:, h * L + o: h * L + o + 512],
                                         in_=xcp[:], func=Act.Silu,
                                         bias=cdb[:, h:h + 1], scale=1.0)
                    nc.scalar.activation(out=sz_big[:, h * L + o: h * L + o + 512],
                                         in_=zp[:], func=Act.Silu, scale=1.0)

                # ============ PHASE 4: x_proj -> (dt_in, B, C) ==============
                xdp = fe.tile([40, 512], FP, tag="ip")
                for kh in range(2):
                    nc.tensor.matmul(
                        out=xdp[:], lhsT=xpw[:, kh * 40:(kh + 1) * 40],
                        rhs=xs_big[:, kh * L + o: kh * L + o + 512],
                        start=(kh == 0), stop=(kh == 1))
                nc.vector.tensor_copy(out=xdbl_sb[:, o:o + 512], in_=xdp[0:40, :])

            # ========== PHASE 5: replicate B, C across partitions ===========
            for n in range(DS):
                nc.sync.dma_start(out=brep[:, n * L:(n + 1) * L],
                                  in_=_row_bcast(xdbl_sb, 8 + n, L))
                nc.sync.dma_start(out=crep[:, n * L:(n + 1) * L],
                                  in_=_row_bcast(xdbl_sb, 24 + n, L))

            # ================= PHASE 6: dt (softplus) + u = dt*xs ===========
            for h in range(2):
                for o in (0, 512):
                    dtp = fe.tile([128, 512], FP, tag="ip")
                    nc.tensor.matmul(
                        out=dtp[:], lhsT=dtw[0:8, h * 128:(h + 1) * 128],
                        rhs=xdbl_sb[0:8, o:o + 512], start=True, stop=True)
                    nc.scalar.activation(
                        out=dt_big[:, h * L + o: h * L + o + 512], in_=dtp[:],
                        func=Act.Exp, bias=dtb[:, h:h + 1], scale=1.0)
            for c0 in (0, 1024):
                nc.scalar.activation(out=dt_big[:, c0:c0 + 1024],
                                     in_=dt_big[:, c0:c0 + 1024],
                                     func=Act.Ln, bias=1.0, scale=1.0)
            for c0 in (0, 1024):
                nc.vector.tensor_mul(out=u_big[:, c0:c0 + 1024],
                                     in0=dt_big[:, c0:c0 + 1024],
                                     in1=xs_big[:, c0:c0 + 1024])

        # ================= PHASE 7: selective scan ==========================
        with tc.tile_pool(name="yp", bufs=1, space="PSUM") as ypp:
            yp = [ypp.tile([128, L], FP, name=f"yp{h}") for h in range(2)]

            def issue_exp(n):
                # dA = exp(A_n * dt); boundary col L stays 0 (resets h at the
                # half-1 start); half-1 cols shifted by one (t=0 dA unused).
                dA = dA_r[n % NB]
                nc.scalar.activation(out=dA[:, 0:L], in_=dt_big[:, 0:L],
                                     func=Act.Exp, scale=asc[:, n:n + 1])
                nc.scalar.activation(out=dA[:, L + 1:2 * L],
                                     in_=dt_big[:, L + 1:2 * L],
                                     func=Act.Exp, scale=asc[:, DS + n:DS + n + 1])

            issue_exp(0)
            issue_exp(1)
            for n in range(DS):
                dA, dBu, ht, hC = (dA_r[n % NB], dBu_r[n % NB],
                                   ht_r[n % NB], hC_r[n % NB])
                nc.vector.tensor_mul(out=dBu[:], in0=u_big[:],
                                     in1=_rep2(brep, n * L, L))
                if n + 2 < DS:
                    issue_exp(n + 2)
                nc.vector.tensor_tensor_scan(
                    out=ht[:], data0=dA[:], data1=dBu[:],
                    initial=0.0, op0=Alu.mult, op1=Alu.add)
                nc.vector.tensor_mul(out=hC[:], in0=ht[:],
                                     in1=_rep2(crep, n * L, L))
                for h in range(2):
                    for oo in (0, 512):
                        nc.tensor.matmul(
                            out=yp[h][:, oo:oo + 512], lhsT=ident[:],
                            rhs=hC[:, h * L + oo: h * L + oo + 512],
                            start=(n == 0), stop=(n == DS - 1))

            # ============= PHASE 8: gate, mean, out_proj, fc ================
            ysum = W.tile([128, 2], FP, name="ysum")
            y1 = [W.tile([128, L], BF, name=f"y1_{h}") for h in range(2)]
            y2 = [W.tile([128, L], BF, name=f"y2_{h}") for h in range(2)]
            for h in range(2):
                nc.vector.scalar_tensor_tensor(
                    out=y1[h][:], in0=xs_big[:, h * L:(h + 1) * L],
                    scalar=dvec[:, h:h + 1], in1=yp[h][:],
                    op0=Alu.mult, op1=Alu.add)
                nc.vector.scalar_tensor_tensor(
                    out=y2[h][:], in0=y1[h][:], scalar=1.0,
                    in1=sz_big[:, h * L:(h + 1) * L],
                    op0=Alu.mult, op1=Alu.mult,
                    accum_out=ysum[:, h:h + 1])

            yop = ypp.tile([128, 1], FP, name="yop")
            for h in range(2):
                nc.tensor.matmul(out=yop[:], lhsT=opw[:, h * 128:(h + 1) * 128],
                                 rhs=ysum[:, h:h + 1], start=(h == 0),
                                 stop=(h == 1))
            ymean = W.tile([128, 1], FP, name="ymean")
            nc.vector.tensor_copy(out=ymean[:], in_=yop[:])
            fcp = ypp.tile([10, 1], FP, name="fcp")
            nc.tensor.matmul(out=fcp[:], lhsT=fcw[:, 0:NCLS], rhs=ymean[:],
                             start=True, stop=True)
            out_sb = W.tile([10, 1], FP, name="out_sb")
            nc.vector.tensor_scalar_add(out=out_sb[:], in0=fcp[:],
                                        scalar1=fcb[0:10, 0:1])
        out_dst = bass.AP(tensor=d_out[:].tensor, offset=0, ap=[[1, NCLS]])
        out_src = bass.AP(tensor=out_sb[:].tensor, offset=out_sb[:].offset,
                          ap=[[out_sb[:].ap[0][0], NCLS]])
        nc.sync.dma_start(out=out_dst, in_=out_src)

    nc.compile()
    return nc


def prep_consts(inputs):
    """Host-side weight transforms (parameters only)."""
    f32 = np.float32
    bf16 = ml_dtypes.bfloat16
    emb = np.ascontiguousarray(np.asarray(inputs["emb"], f32).astype(bf16))
    conv1_w = np.asarray(inputs["conv1_w"], f32)      # (128, 256, 5)
    conv1_b = np.asarray(inputs["conv1_b"], f32)
    in_proj_w = np.asarray(inputs["in_proj_w"], f32)  # (512, 128)
    convd_w = np.asarray(inputs["convd_w"], f32)      # (256, 1, 4)
    convd_b = np.asarray(inputs["convd_b"], f32)
    x_proj_w = np.asarray(inputs["x_proj_w"], f32)    # (40, 256)
    dt_proj_w = np.asarray(inputs["dt_proj_w"], f32)  # (256, 8)
    dt_proj_b = np.asarray(inputs["dt_proj_b"], f32)
    A_log = np.asarray(inputs["A_log"], f32)          # (256, 16)
    Dv = np.asarray(inputs["D"], f32)
    out_proj_w = np.asarray(inputs["out_proj_w"], f32)  # (128, 256)
    fc_w = np.asarray(inputs["fc_w"], f32)            # (10, 128)
    fc_b = np.asarray(inputs["fc_b"], f32)

    c1w = np.zeros((128, 5, 2, 128), f32)
    for k in range(5):
        for kh in range(2):
            c1w[:, k, kh, :] = conv1_w[:, kh * 128:(kh + 1) * 128, k].T
    c1w = c1w.reshape(128, -1)

    Wx = in_proj_w[:DI]          # (256, 128)
    xcw = np.zeros((128, 4, 2, 128), f32)
    for k in range(4):
        Wxk = convd_w[:, 0, k][:, None] * Wx          # (256, 128)
        for mc in range(2):
            xcw[:, k, mc, :] = Wxk[mc * 128:(mc + 1) * 128, :].T
    xcw = xcw.reshape(128, -1)

    Wz = in_proj_w[DI:]
    zw = np.zeros((128, 2, 128), f32)
    for mc in range(2):
        zw[:, mc, :] = Wz[mc * 128:(mc + 1) * 128, :].T
    zw = zw.reshape(128, -1)

    xpw = np.zeros((128, 2, 40), f32)
    for kh in range(2):
        xpw[:, kh, :] = x_proj_w[:, kh * 128:(kh + 1) * 128].T
    xpw = xpw.reshape(128, -1)

    dtw = np.zeros((8, 2, 128), f32)
    for mc in range(2):
        dtw[:, mc, :] = dt_proj_w[mc * 128:(mc + 1) * 128, :].T
    dtw = dtw.reshape(8, -1)

    A = -np.exp(A_log)           # (256, 16)
    asc = np.zeros((128, 2 * DS), f32)
    asc[:, :DS] = A[:128]        # half-0 channels
    asc[:, DS:] = A[128:]        # half-1 channels

    opw = np.zeros((128, 2, 128), f32)
    for kh in range(2):
        opw[:, kh, :] = out_proj_w[:, kh * 128:(kh + 1) * 128].T
    opw = opw.reshape(128, -1)

    fcw = (fc_w / float(L)).T.copy()                  # (128, 10)

    consts = {
        "emb": emb,
        "c1w": c1w.astype(bf16), "xcw": xcw.astype(bf16), "zw": zw.astype(bf16),
        "xpw": xpw.astype(bf16), "dtw": dtw.astype(bf16),
        "asc": asc, "opw": opw, "fcw": fcw,
        "ident": np.eye(128, dtype=f32).astype(bf16),
        "c1b": conv1_b.reshape(128, 1).copy(),
        "cdb": convd_b.reshape(2, 128).T.copy(),
        "dtb": dt_proj_b.reshape(2, 128).T.copy(),
        "dvec": Dv.reshape(2, 128).T.copy(),
        "fcb": fc_b.reshape(10, 1).copy(),
    }
    return consts


_CACHE = {}


def kernel(**inputs) -> np.ndarray:
    ids = np.asarray(inputs["ids"])
    assert ids.shape == (8, SEQ), ids.shape
    ids32 = np.ascontiguousarray(ids, dtype=np.int32)

    if "nc" not in _CACHE:
        _CACHE["nc"] = build_program()
    nc = _CACHE["nc"]
    nonce_name = [t for t in (a.memorylocations[0].name
                              for a in nc.m.functions[0].allocations
                              if getattr(a, "kind", None) == "ExternalInput"
                              and a.memorylocations)
                  if t.startswith("nonce_")][0]

    consts = prep_consts(inputs)
    in_maps = []
    for b in range(8):
        m = dict(consts)
        m["ids"] = np.ascontiguousarray(ids32[b].reshape(16, 128).T)
        m[nonce_name] = np.zeros((1, 1), np.float32)
        in_maps.append(m)

    trace = os.environ.get("MAMBA_TRACE", "0") == "1"
    res = run_bass_kernel_spmd(nc, in_maps, core_ids=list(range(8)), trace=trace)
    _CACHE["last_results"] = res
    out = np.stack([res.results[b]["out"] for b in range(8)]).astype(np.float32)
    return out


# revision 23
# speedup vs baseline: 1.3436x; 1.3436x over previous
"""Trainium2 Bass kernel for CNN+Mamba classifier.

Contract: kernel(**inputs) takes FULL unsharded inputs (numpy), returns FULL
(8, 10) float32 output. Internally shards data-parallel over batch across 8
NeuronCores (1 example per core), with all parameters replicated.

v3 architecture:
  - Embedding gather via gpsimd indirect DMA + PE transposes (proven path).
  - Selective scan tiled channel-major: one [128, 2048] tile per state n
    (both 128-channel halves merged along the free dim; dA boundary column
    kept 0 so the recurrence resets between halves). No replication matmuls,
    no PSUM->SBUF copies.
  - B/C rows replicated across partitions by DMA; read twice via 0-stride AP.
  - All scan-phase elementwise work on DVE (GpSimd shares SBUF ports with
    DVE, so offloading there is a wash); exps on Act; y accumulated over
    states via identity-lhsT PSUM matmuls; mean folded into stt accum_out.

Self-contained: hardcodes all shapes; no sibling imports.
"""

import os
from contextlib import ExitStack

import numpy as np
import ml_dtypes

import concourse.bass as bass
import concourse.bacc as bacc
import concourse.tile as tile
from concourse import mybir
from concourse.bass_utils import run_bass_kernel_spmd

FP = mybir.dt.float32
BF = mybir.dt.bfloat16
I32 = mybir.dt.int32

VOCAB, EMB, NCLS, SEQ = 50000, 256, 10, 2048
DM, DI, DS, DCONV, DTR = 128, 256, 16, 4, 8
L = SEQ // 2  # 1024 after maxpool


def _rep2(t_ap, col0, n):
    """in1 AP reading cols [col0, col0+n) of a [128, *] tile twice (0-stride)."""
    full = t_ap[:]
    return bass.AP(tensor=full.tensor, offset=full.offset + col0,
                   ap=[[full.ap[0][0], 128], [0, 2], [1, n]])


def _row_bcast(t_ap, row, n):
    """DMA source AP: row `row` of tile, broadcast to 128 partitions."""
    full = t_ap[:]
    pstep = full.ap[0][0]
    return bass.AP(tensor=full.tensor, offset=full.offset + row * pstep,
                   ap=[[pstep, 1], [0, 128], [1, n]])


def build_program():
    nc = bacc.Bacc("TRN2", target_bir_lowering=False, debug=False, num_devices=8)

    d_ids = nc.dram_tensor("ids", [128, 16], I32, kind="ExternalInput")
    d_emb = nc.dram_tensor("emb", [VOCAB, EMB], BF, kind="ExternalInput")
    d_c1w = nc.dram_tensor("c1w", [128, 5 * 2 * 128], BF, kind="ExternalInput")
    d_xcw = nc.dram_tensor("xcw", [128, 4 * 2 * 128], BF, kind="ExternalInput")
    d_zw = nc.dram_tensor("zw", [128, 2 * 128], BF, kind="ExternalInput")
    d_xpw = nc.dram_tensor("xpw", [128, 2 * 40], BF, kind="ExternalInput")
    d_dtw = nc.dram_tensor("dtw", [8, 2 * 128], BF, kind="ExternalInput")
    d_asc = nc.dram_tensor("asc", [128, 2 * DS], FP, kind="ExternalInput")
    d_opw = nc.dram_tensor("opw", [128, 2 * 128], FP, kind="ExternalInput")
    d_fcw = nc.dram_tensor("fcw", [128, NCLS], FP, kind="ExternalInput")
    d_ident = nc.dram_tensor("ident", [128, 128], BF, kind="ExternalInput")
    d_c1b = nc.dram_tensor("c1b", [128, 1], FP, kind="ExternalInput")
    d_cdb = nc.dram_tensor("cdb", [128, 2], FP, kind="ExternalInput")
    d_dtb = nc.dram_tensor("dtb", [128, 2], FP, kind="ExternalInput")
    d_dvec = nc.dram_tensor("dvec", [128, 2], FP, kind="ExternalInput")
    d_fcb = nc.dram_tensor("fcb", [10, 1], FP, kind="ExternalInput")

    import uuid
    nonce = uuid.uuid4().hex[:12]
    d_nonce = nc.dram_tensor(f"nonce_{nonce}", [1, 1], FP, kind="ExternalInput")
    d_out = nc.dram_tensor("out", [NCLS], FP, kind="ExternalOutput")

    Alu = mybir.AluOpType
    Act = mybir.ActivationFunctionType

    with ExitStack() as ctx:
        tc = ctx.enter_context(tile.TileContext(nc))
        W = ctx.enter_context(tc.tile_pool(name="w", bufs=1))
        nonce_sb = W.tile([1, 1], FP, name="nonce_sb")
        nc.sync.dma_start(out=nonce_sb[:], in_=d_nonce[:])

        def load(dram, shape, dtype=FP):
            t = W.tile(list(shape), dtype, name=f"w_{dram.name}")
            nc.sync.dma_start(out=t[:], in_=dram[:])
            return t

        ids_sb = W.tile([128, 16], I32, name="ids_sb")
        nc.sync.dma_start(out=ids_sb[:], in_=d_ids[:])
        c1w = load(d_c1w, (128, 5 * 2 * 128), BF)
        xcw = load(d_xcw, (128, 4 * 2 * 128), BF)
        zw = load(d_zw, (128, 2 * 128), BF)
        xpw = load(d_xpw, (128, 2 * 40), BF)
        dtw = load(d_dtw, (8, 2 * 128), BF)
        asc = load(d_asc, (128, 2 * DS))
        opw = load(d_opw, (128, 2 * 128))
        fcw = load(d_fcw, (128, NCLS))
        ident = load(d_ident, (128, 128), BF)
        c1b = load(d_c1b, (128, 1))
        cdb = load(d_cdb, (128, 2))
        dtb = load(d_dtb, (128, 2))
        dvec = load(d_dvec, (128, 2))
        fcb = load(d_fcb, (10, 1))

        # ---- persistent intermediates ----
        x_emb = [W.tile([128, SEQ + 4], BF, name=f"x_emb{_}") for _ in range(2)]
        for h in range(2):
            nc.vector.memset(x_emb[h][:, 0:2], 0.0)
            nc.vector.memset(x_emb[h][:, SEQ + 2:SEQ + 4], 0.0)
        relu_sb = W.tile([128, SEQ], BF, name="relu_sb")
        x_pool = W.tile([128, L + 3], BF, name="x_pool")
        nc.vector.memset(x_pool[:, 0:3], 0.0)
        xs_big = W.tile([128, 2 * L], BF, name="xs_big")
        sz_big = W.tile([128, 2 * L], BF, name="sz_big")
        dt_big = W.tile([128, 2 * L], FP, name="dt_big")
        u_big = W.tile([128, 2 * L], BF, name="u_big")
        xdbl_sb = W.tile([40, L], BF, name="xdbl_sb")
        brep = W.tile([128, DS * L], BF, name="brep")
        crep = W.tile([128, DS * L], BF, name="crep")

        # scan working rings; dA col L is the recurrence reset (stays 0)
        NB = 4
        NBA = 6
        dA_r = [W.tile([128, 2 * L], BF, name=f"dA{i}") for i in range(NBA)]
        for i in range(NBA):
            nc.vector.memset(dA_r[i][:, L:L + 1], 0.0)
        dBu_r = [W.tile([128, 2 * L], BF, name=f"dBu{i}") for i in range(NB)]
        ht_r = [W.tile([128, 2 * L], BF, name=f"ht{i}") for i in range(NB)]
        hC_r = [W.tile([128, 2 * L], BF, name=f"hC{i}") for i in range(NB)]

        # preload the silu ACT table (first real Act op is silu)
        scratch = W.tile([128, 2], FP, name="act_scratch")
        nc.vector.memset(scratch[:], 1.0)
        nc.scalar.activation(out=scratch[:, 0:1], in_=scratch[:, 0:1],
                             func=Act.Silu, scale=1.0)

        # ============ PHASE 1+2: gather + transpose | conv + relu + max =====
        with tc.tile_pool(name="g", bufs=8) as gp, \
             tc.tile_pool(name="gt", bufs=2, space="PSUM") as gtp, \
             tc.tile_pool(name="wrm", bufs=1, space="PSUM") as wrm, \
             tc.tile_pool(name="cps", bufs=2, space="PSUM") as cpp, \
             tc.tile_pool(name="fe", bufs=3, space="PSUM") as fe:

            def conv_group(g):
                o = 512 * g
                cps = cpp.tile([128, 512], FP, tag="cps")
                for k in range(5):
                    for kh in range(2):
                        nc.tensor.matmul(
                            out=cps[:],
                            lhsT=c1w[:, (k * 2 + kh) * 128:(k * 2 + kh + 1) * 128],
                            rhs=x_emb[kh][:, o + k: o + k + 512],
                            start=(k == 0 and kh == 0), stop=(k == 4 and kh == 1))
                nc.vector.tensor_scalar(
                    out=relu_sb[:, o:o + 512], in0=cps[:],
                    scalar1=c1b[:, 0:1], scalar2=0.0, op0=Alu.add, op1=Alu.max)
                rf = relu_sb[:]
                ev = bass.AP(tensor=rf.tensor, offset=rf.offset + o,
                             ap=[[rf.ap[0][0], 128], [2, 256]])
                od = bass.AP(tensor=rf.tensor, offset=rf.offset + o + 1,
                             ap=[[rf.ap[0][0], 128], [2, 256]])
                nc.vector.tensor_max(out=x_pool[:, 3 + o // 2: 3 + o // 2 + 256],
                                     in0=ev, in1=od)

            wps = wrm.tile([128, 512], FP, name="warm_ps")
            for c in range(16):
                xg = gp.tile([128, EMB], BF, tag="xg")
                nc.gpsimd.indirect_dma_start(
                    out=xg[:], out_offset=None, in_=d_emb[:],
                    in_offset=bass.IndirectOffsetOnAxis(ap=ids_sb[:, c:c + 1],
                                                        axis=0))
                for h in range(2):
                    pt = gtp.tile([128, 128], BF, tag="pt")
                    nc.tensor.transpose(out=pt[:], in_=xg[:, 128 * h:128 * (h + 1)],
                                        identity=ident[:])
                    nc.vector.tensor_copy(
                        out=x_emb[h][:, 2 + 128 * c:2 + 128 * (c + 1)], in_=pt[:])
                if 2 <= c < 10:
                    # PE pstate warmup while the gather streams in
                    nc.tensor.matmul(out=wps[:], lhsT=c1w[:, 0:128],
                                     rhs=c1w[:, 0:512], start=True, stop=True)
                # issue conv group g once chunks 0..4g+4 are transposed
                if c >= 4 and c % 4 == 0:
                    conv_group(c // 4 - 1)
            conv_group(3)

            # ====== PHASE 3: in_proj (+ folded depthwise conv) + silu =======
            for o in (0, 512):
                zps = []
                for h in range(2):
                    xcp = fe.tile([128, 512], FP, tag="ip")
                    for k in range(4):
                        nc.tensor.matmul(
                            out=xcp[:],
                            lhsT=xcw[:, (k * 2 + h) * 128:(k * 2 + h + 1) * 128],
                            rhs=x_pool[:, o + k: o + k + 512],
                            start=(k == 0), stop=(k == 3))
                    zp = fe.tile([128, 512], FP, tag="ip")
                    nc.tensor.matmul(
                        out=zp[:], lhsT=zw[:, h * 128:(h + 1) * 128],
                        rhs=x_pool[:, 3 + o: 3 + o + 512], start=True, stop=True)
                    zps.append(zp)
                    # xs silu first: it gates x_proj -> dt -> scan start
                    nc.scalar.activation(out=xs_big[:, h * L + o: h * L + o + 512],
                                         in_=xcp[:], func=Act.Silu,
                                         bias=cdb[:, h:h + 1], scale=1.0)

                # ============ PHASE 4: x_proj -> (dt_in, B, C) ==============
                xdp = fe.tile([40, 512], FP, tag="ip")
                for kh in range(2):
                    nc.tensor.matmul(
                        out=xdp[:], lhsT=xpw[:, kh * 40:(kh + 1) * 40],
                        rhs=xs_big[:, kh * L + o: kh * L + o + 512],
                        start=(kh == 0), stop=(kh == 1))
                nc.vector.tensor_copy(out=xdbl_sb[:, o:o + 512], in_=xdp[0:40, :])
                for h in range(2):
                    nc.scalar.activation(out=sz_big[:, h * L + o: h * L + o + 512],
                                         in_=zps[h][:], func=Act.Silu, scale=1.0)

            # ========== PHASE 5: replicate B, C across partitions ===========
            # Two-stage tree: single-row broadcast DMAs bottleneck on the one
            # source partition's read port (~5us per row). Stage 1 makes 8
            # copies of each row spread across 128 partitions (reads spread
            # over 16 source partitions); stage 2 reads 8 partitions 16x each.
            xf = xdbl_sb[:]
            xstep = xf.ap[0][0]

            def stage1(dst, row0):
                src = bass.AP(tensor=xf.tensor, offset=xf.offset + row0 * xstep,
                              ap=[[xstep, 16], [0, 8], [1, L]])
                nc.sync.dma_start(out=dst[:], in_=src)

            btmp = W.tile([128, L], BF, name="btmp")
            ctmp = W.tile([128, L], BF, name="ctmp")
            stage1(btmp, 8)
            stage1(ctmp, 24)

            def stage2(dst_pool, tmp, n):
                tf = tmp[:]
                tstep = tf.ap[0][0]
                src = bass.AP(tensor=tf.tensor,
                              offset=tf.offset + (n * 8) * tstep,
                              ap=[[tstep, 8], [0, 16], [1, L]])
                nc.sync.dma_start(out=dst_pool[:, n * L:(n + 1) * L], in_=src)

            def stage2_gp(dst_pool, tmp, n):
                tf = tmp[:]
                tstep = tf.ap[0][0]
                src = bass.AP(tensor=tf.tensor,
                              offset=tf.offset + (n * 8) * tstep,
                              ap=[[tstep, 8], [0, 16], [1, L]])
                nc.gpsimd.dma_start(out=dst_pool[:, n * L:(n + 1) * L], in_=src)

            for n in range(DS):
                stage2_gp(brep, btmp, n)   # SWDGE queue (Pool idle here)
                stage2(crep, ctmp, n)      # SP queue

            # ================= PHASE 6: dt (softplus) + u = dt*xs ===========
            for h in range(2):
                for o in (0, 512):
                    dtp = fe.tile([128, 512], FP, tag="ip")
                    nc.tensor.matmul(
                        out=dtp[:], lhsT=dtw[0:8, h * 128:(h + 1) * 128],
                        rhs=xdbl_sb[0:8, o:o + 512], start=True, stop=True)
                    nc.scalar.activation(
                        out=dt_big[:, h * L + o: h * L + o + 512], in_=dtp[:],
                        func=Act.Exp, bias=dtb[:, h:h + 1], scale=1.0)
            for c0 in (0, 1024):
                nc.scalar.activation(out=dt_big[:, c0:c0 + 1024],
                                     in_=dt_big[:, c0:c0 + 1024],
                                     func=Act.Ln, bias=1.0, scale=1.0)
            for c0 in (0, 1024):
                nc.vector.tensor_mul(out=u_big[:, c0:c0 + 1024],
                                     in0=dt_big[:, c0:c0 + 1024],
                                     in1=xs_big[:, c0:c0 + 1024])


        # ================= PHASE 7: selective scan ==========================
        with tc.tile_pool(name="yp", bufs=1, space="PSUM") as ypp:
            yp = [ypp.tile([128, L], FP, name=f"yp{h}") for h in range(2)]

            def issue_exp(n):
                # dA = exp(A_n * dt); boundary col L stays 0 (resets h at the
                # half-1 start); half-1 cols shifted by one (t=0 dA unused).
                dA = dA_r[n % NBA]
                nc.scalar.activation(out=dA[:, 0:L], in_=dt_big[:, 0:L],
                                     func=Act.Exp, scale=asc[:, n:n + 1])
                nc.scalar.activation(out=dA[:, L + 1:2 * L],
                                     in_=dt_big[:, L + 1:2 * L],
                                     func=Act.Exp, scale=asc[:, DS + n:DS + n + 1])

            for i in range(4):
                issue_exp(i)
            for n in range(DS):
                dA, dBu, ht, hC = (dA_r[n % NBA], dBu_r[n % NB],
                                   ht_r[n % NB], hC_r[n % NB])
                nc.vector.tensor_mul(out=dBu[:], in0=u_big[:],
                                     in1=_rep2(brep, n * L, L))
                if n + 4 < DS:
                    issue_exp(n + 4)
                nc.vector.tensor_tensor_scan(
                    out=ht[:], data0=dA[:], data1=dBu[:],
                    initial=0.0, op0=Alu.mult, op1=Alu.add)
                nc.vector.tensor_mul(out=hC[:], in0=ht[:],
                                     in1=_rep2(crep, n * L, L))
                for h in range(2):
                    for oo in (0, 512):
                        nc.tensor.matmul(
                            out=yp[h][:, oo:oo + 512], lhsT=ident[:],
                            rhs=hC[:, h * L + oo: h * L + oo + 512],
                            start=(n == 0), stop=(n == DS - 1))

            # ============= PHASE 8: gate, mean, out_proj, fc ================
            ysum = W.tile([128, 2], FP, name="ysum")
            y1 = [W.tile([128, L], BF, name=f"y1_{h}") for h in range(2)]
            y2 = [W.tile([128, L], BF, name=f"y2_{h}") for h in range(2)]
            for h in range(2):
                nc.vector.scalar_tensor_tensor(
                    out=y1[h][:], in0=xs_big[:, h * L:(h + 1) * L],
                    scalar=dvec[:, h:h + 1], in1=yp[h][:],
                    op0=Alu.mult, op1=Alu.add)
                nc.vector.scalar_tensor_tensor(
                    out=y2[h][:], in0=y1[h][:], scalar=1.0,
                    in1=sz_big[:, h * L:(h + 1) * L],
                    op0=Alu.mult, op1=Alu.mult,
                    accum_out=ysum[:, h:h + 1])

            yop = ypp.tile([128, 1], FP, name="yop")
            for h in range(2):
                nc.tensor.matmul(out=yop[:], lhsT=opw[:, h * 128:(h + 1) * 128],
                                 rhs=ysum[:, h:h + 1], start=(h == 0),
                                 stop=(h == 1))
            ymean = W.tile([128, 1], FP, name="ymean")
            nc.vector.tensor_copy(out=ymean[:], in_=yop[:])
            fcp = ypp.tile([10, 1], FP, name="fcp")
            nc.tensor.matmul(out=fcp[:], lhsT=fcw[:, 0:NCLS], rhs=ymean[:],
                             start=True, stop=True)
            out_sb = W.tile([10, 1], FP, name="out_sb")
            nc.vector.tensor_scalar_add(out=out_sb[:], in0=fcp[:],
                                        scalar1=fcb[0:10, 0:1])
        out_dst = bass.AP(tensor=d_out[:].tensor, offset=0, ap=[[1, NCLS]])
        out_src = bass.AP(tensor=out_sb[:].tensor, offset=out_sb[:].offset,
                          ap=[[out_sb[:].ap[0][0], NCLS]])
        nc.sync.dma_start(out=out_dst, in_=out_src)

    import concourse.bacc as _bacc
    _orig_gat = _bacc.get_activation_tables

    def _gat(arch):
        t = dict(_orig_gat(arch))
        if "natural_log_exp_and_others" in t:
            t2 = {"natural_log_exp_and_others": t.pop("natural_log_exp_and_others")}
            t2.update(t)
            return t2
        return t

    _bacc.get_activation_tables = _gat
    try:
        nc.compile()
    finally:
        _bacc.get_activation_tables = _orig_gat
    return nc


def prep_consts(inputs):
    """Host-side weight transforms (parameters only)."""
    f32 = np.float32
    bf16 = ml_dtypes.bfloat16
    emb = np.ascontiguousarray(np.asarray(inputs["emb"], f32).astype(bf16))
    conv1_w = np.asarray(inputs["conv1_w"], f32)      # (128, 256, 5)
    conv1_b = np.asarray(inputs["conv1_b"], f32)
    in_proj_w = np.asarray(inputs["in_proj_w"], f32)  # (512, 128)
    convd_w = np.asarray(inputs["convd_w"], f32)      # (256, 1, 4)
    convd_b = np.asarray(inputs["convd_b"], f32)
    x_proj_w = np.asarray(inputs["x_proj_w"], f32)    # (40, 256)
    dt_proj_w = np.asarray(inputs["dt_proj_w"], f32)  # (256, 8)
    dt_proj_b = np.asarray(inputs["dt_proj_b"], f32)
    A_log = np.asarray(inputs["A_log"], f32)          # (256, 16)
    Dv = np.asarray(inputs["D"], f32)
    out_proj_w = np.asarray(inputs["out_proj_w"], f32)  # (128, 256)
    fc_w = np.asarray(inputs["fc_w"], f32)            # (10, 128)
    fc_b = np.asarray(inputs["fc_b"], f32)

    c1w = np.zeros((128, 5, 2, 128), f32)
    for k in range(5):
        for kh in range(2):
            c1w[:, k, kh, :] = conv1_w[:, kh * 128:(kh + 1) * 128, k].T
    c1w = c1w.reshape(128, -1)

    Wx = in_proj_w[:DI]          # (256, 128)
    xcw = np.zeros((128, 4, 2, 128), f32)
    for k in range(4):
        Wxk = convd_w[:, 0, k][:, None] * Wx          # (256, 128)
        for mc in range(2):
            xcw[:, k, mc, :] = Wxk[mc * 128:(mc + 1) * 128, :].T
    xcw = xcw.reshape(128, -1)

    Wz = in_proj_w[DI:]
    zw = np.zeros((128, 2, 128), f32)
    for mc in range(2):
        zw[:, mc, :] = Wz[mc * 128:(mc + 1) * 128, :].T
    zw = zw.reshape(128, -1)

    xpw = np.zeros((128, 2, 40), f32)
    for kh in range(2):
        xpw[:, kh, :] = x_proj_w[:, kh * 128:(kh + 1) * 128].T
    xpw = xpw.reshape(128, -1)

    dtw = np.zeros((8, 2, 128), f32)
    for mc in range(2):
        dtw[:, mc, :] = dt_proj_w[mc * 128:(mc + 1) * 128, :].T
    dtw = dtw.reshape(8, -1)

    A = -np.exp(A_log)           # (256, 16)
    asc = np.zeros((128, 2 * DS), f32)
    asc[:, :DS] = A[:128]        # half-0 channels
    asc[:, DS:] = A[128:]        # half-1 channels

    opw = np.zeros((128, 2, 128), f32)
    for kh in range(2):
        opw[:, kh, :] = out_proj_w[:, kh * 128:(kh + 1) * 128].T
    opw = opw.reshape(128, -1)

    fcw = (fc_w / float(L)).T.copy()                  # (128, 10)

    consts = {
        "emb": emb,
        "c1w": c1w.astype(bf16), "xcw": xcw.astype(bf16), "zw": zw.astype(bf16),
        "xpw": xpw.astype(bf16), "dtw": dtw.astype(bf16),
        "asc": asc, "opw": opw, "fcw": fcw,
        "ident": np.eye(128, dtype=f32).astype(bf16),
        "c1b": conv1_b.reshape(128, 1).copy(),
        "cdb": convd_b.reshape(2, 128).T.copy(),
        "dtb": dt_proj_b.reshape(2, 128).T.copy(),
        "dvec": Dv.reshape(2, 128).T.copy(),
        "fcb": fc_b.reshape(10, 1).copy(),
    }
    return consts


_CACHE = {}


def kernel(**inputs) -> np.ndarray:
    ids = np.asarray(inputs["ids"])
    assert ids.shape == (8, SEQ), ids.shape
    ids32 = np.ascontiguousarray(ids, dtype=np.int32)

    if "nc" not in _CACHE:
        _CACHE["nc"] = build_program()
    nc = _CACHE["nc"]
    nonce_name = [t for t in (a.memorylocations[0].name
                              for a in nc.m.functions[0].allocations
                              if getattr(a, "kind", None) == "ExternalInput"
                              and a.memorylocations)
                  if t.startswith("nonce_")][0]

    consts = prep_consts(inputs)
    in_maps = []
    for b in range(8):
        m = dict(consts)
        m["ids"] = np.ascontiguousarray(ids32[b].reshape(16, 128).T)
        m[nonce_name] = np.zeros((1, 1), np.float32)
        in_maps.append(m)

    trace = os.environ.get("MAMBA_TRACE", "0") == "1"
    res = run_bass_kernel_spmd(nc, in_maps, core_ids=list(range(8)), trace=trace)
    _CACHE["last_results"] = res
    out = np.stack([res.results[b]["out"] for b in range(8)]).astype(np.float32)
    return out


# revision 24
# speedup vs baseline: 1.3520x; 1.0062x over previous
"""Trainium2 Bass kernel for CNN+Mamba classifier.

Contract: kernel(**inputs) takes FULL unsharded inputs (numpy), returns FULL
(8, 10) float32 output. Internally shards data-parallel over batch across 8
NeuronCores (1 example per core), with all parameters replicated.

v3 architecture:
  - Embedding gather via gpsimd indirect DMA + PE transposes (proven path).
  - Selective scan tiled channel-major: one [128, 2048] tile per state n
    (both 128-channel halves merged along the free dim; dA boundary column
    kept 0 so the recurrence resets between halves). No replication matmuls,
    no PSUM->SBUF copies.
  - B/C rows replicated across partitions by DMA; read twice via 0-stride AP.
  - All scan-phase elementwise work on DVE (GpSimd shares SBUF ports with
    DVE, so offloading there is a wash); exps on Act; y accumulated over
    states via identity-lhsT PSUM matmuls; mean folded into stt accum_out.

Self-contained: hardcodes all shapes; no sibling imports.
"""

import os
from contextlib import ExitStack

import numpy as np
import ml_dtypes

import concourse.bass as bass
import concourse.bacc as bacc
import concourse.tile as tile
from concourse import mybir
from concourse.bass_utils import run_bass_kernel_spmd

FP = mybir.dt.float32
BF = mybir.dt.bfloat16
I32 = mybir.dt.int32

VOCAB, EMB, NCLS, SEQ = 50000, 256, 10, 2048
DM, DI, DS, DCONV, DTR = 128, 256, 16, 4, 8
L = SEQ // 2  # 1024 after maxpool


def _rep2(t_ap, col0, n):
    """in1 AP reading cols [col0, col0+n) of a [128, *] tile twice (0-stride)."""
    full = t_ap[:]
    return bass.AP(tensor=full.tensor, offset=full.offset + col0,
                   ap=[[full.ap[0][0], 128], [0, 2], [1, n]])


def _row_bcast(t_ap, row, n):
    """DMA source AP: row `row` of tile, broadcast to 128 partitions."""
    full = t_ap[:]
    pstep = full.ap[0][0]
    return bass.AP(tensor=full.tensor, offset=full.offset + row * pstep,
                   ap=[[pstep, 1], [0, 128], [1, n]])


def build_program():
    nc = bacc.Bacc("TRN2", target_bir_lowering=False, debug=False, num_devices=8)

    d_ids = nc.dram_tensor("ids", [128, 16], I32, kind="ExternalInput")
    d_emb = nc.dram_tensor("emb", [VOCAB, EMB], BF, kind="ExternalInput")
    d_c1w = nc.dram_tensor("c1w", [128, 5 * 2 * 128], BF, kind="ExternalInput")
    d_xcw = nc.dram_tensor("xcw", [128, 4 * 2 * 128], BF, kind="ExternalInput")
    d_zw = nc.dram_tensor("zw", [128, 2 * 128], BF, kind="ExternalInput")
    d_xpw = nc.dram_tensor("xpw", [128, 2 * 40], BF, kind="ExternalInput")
    d_dtw = nc.dram_tensor("dtw", [8, 2 * 128], BF, kind="ExternalInput")
    d_asc = nc.dram_tensor("asc", [128, 2 * DS], FP, kind="ExternalInput")
    d_opw = nc.dram_tensor("opw", [128, 2 * 128], FP, kind="ExternalInput")
    d_fcw = nc.dram_tensor("fcw", [128, NCLS], FP, kind="ExternalInput")
    d_ident = nc.dram_tensor("ident", [128, 128], BF, kind="ExternalInput")
    d_c1b = nc.dram_tensor("c1b", [128, 1], FP, kind="ExternalInput")
    d_cdb = nc.dram_tensor("cdb", [128, 2], FP, kind="ExternalInput")
    d_dtb = nc.dram_tensor("dtb", [128, 2], FP, kind="ExternalInput")
    d_dvec = nc.dram_tensor("dvec", [128, 2], FP, kind="ExternalInput")
    d_ddw = nc.dram_tensor("ddw", [128, 2 * 128], BF, kind="ExternalInput")
    d_fcb = nc.dram_tensor("fcb", [10, 1], FP, kind="ExternalInput")

    import uuid
    nonce = uuid.uuid4().hex[:12]
    d_nonce = nc.dram_tensor(f"nonce_{nonce}", [1, 1], FP, kind="ExternalInput")
    d_out = nc.dram_tensor("out", [NCLS], FP, kind="ExternalOutput")

    Alu = mybir.AluOpType
    Act = mybir.ActivationFunctionType

    with ExitStack() as ctx:
        tc = ctx.enter_context(tile.TileContext(nc))
        W = ctx.enter_context(tc.tile_pool(name="w", bufs=1))
        ids_sb = W.tile([128, 16], I32, name="ids_sb")
        nc.sync.dma_start(out=ids_sb[:], in_=d_ids[:])
        nonce_sb = W.tile([1, 1], FP, name="nonce_sb")
        nc.sync.dma_start(out=nonce_sb[:], in_=d_nonce[:])
        pwarm = W.tile([128, 2], FP, name="pool_warm")
        nc.gpsimd.memset(pwarm[:], 0.0)

        def load(dram, shape, dtype=FP):
            t = W.tile(list(shape), dtype, name=f"w_{dram.name}")
            nc.sync.dma_start(out=t[:], in_=dram[:])
            return t
        c1w = load(d_c1w, (128, 5 * 2 * 128), BF)
        xcw = load(d_xcw, (128, 4 * 2 * 128), BF)
        zw = load(d_zw, (128, 2 * 128), BF)
        xpw = load(d_xpw, (128, 2 * 40), BF)
        dtw = load(d_dtw, (8, 2 * 128), BF)
        asc = load(d_asc, (128, 2 * DS))
        opw = load(d_opw, (128, 2 * 128))
        fcw = load(d_fcw, (128, NCLS))
        ident = load(d_ident, (128, 128), BF)
        c1b = load(d_c1b, (128, 1))
        cdb = load(d_cdb, (128, 2))
        dtb = load(d_dtb, (128, 2))
        dvec = load(d_dvec, (128, 2))
        ddw = load(d_ddw, (128, 2 * 128), BF)
        fcb = load(d_fcb, (10, 1))

        # ---- persistent intermediates ----
        x_emb = [W.tile([128, SEQ + 4], BF, name=f"x_emb{_}") for _ in range(2)]
        for h in range(2):
            nc.vector.memset(x_emb[h][:, 0:2], 0.0)
            nc.vector.memset(x_emb[h][:, SEQ + 2:SEQ + 4], 0.0)
        relu_sb = W.tile([128, SEQ], BF, name="relu_sb")
        x_pool = W.tile([128, L + 3], BF, name="x_pool")
        nc.vector.memset(x_pool[:, 0:3], 0.0)
        xs_big = W.tile([128, 2 * L], BF, name="xs_big")
        sz_big = W.tile([128, 2 * L], BF, name="sz_big")
        dt_big = W.tile([128, 2 * L], FP, name="dt_big")
        u_big = W.tile([128, 2 * L], BF, name="u_big")
        xdbl_sb = W.tile([40, L], BF, name="xdbl_sb")
        brep = W.tile([128, DS * L], BF, name="brep")
        crep = W.tile([128, DS * L], BF, name="crep")

        # scan working rings; dA col L is the recurrence reset (stays 0)
        NB = 4
        NBA = 6
        dA_r = [W.tile([128, 2 * L], BF, name=f"dA{i}") for i in range(NBA)]
        for i in range(NBA):
            nc.vector.memset(dA_r[i][:, L:L + 1], 0.0)
        dBu_r = [W.tile([128, 2 * L], BF, name=f"dBu{i}") for i in range(NB)]
        ht_r = [W.tile([128, 2 * L], BF, name=f"ht{i}") for i in range(NB)]
        hC_r = [W.tile([128, 2 * L], BF, name=f"hC{i}") for i in range(NB)]

        # preload the silu ACT table (first real Act op is silu)
        scratch = W.tile([128, 2], FP, name="act_scratch")
        nc.vector.memset(scratch[:], 1.0)
        nc.scalar.activation(out=scratch[:, 0:1], in_=scratch[:, 0:1],
                             func=Act.Silu, scale=1.0)

        # ============ PHASE 1+2: gather + transpose | conv + relu + max =====
        with tc.tile_pool(name="g", bufs=8) as gp, \
             tc.tile_pool(name="gt", bufs=2, space="PSUM") as gtp, \
             tc.tile_pool(name="wrm", bufs=1, space="PSUM") as wrm, \
             tc.tile_pool(name="cps", bufs=2, space="PSUM") as cpp, \
             tc.tile_pool(name="fe", bufs=3, space="PSUM") as fe:

            def conv_group(g):
                o = 512 * g
                cps = cpp.tile([128, 512], FP, tag="cps")
                for k in range(5):
                    for kh in range(2):
                        nc.tensor.matmul(
                            out=cps[:],
                            lhsT=c1w[:, (k * 2 + kh) * 128:(k * 2 + kh + 1) * 128],
                            rhs=x_emb[kh][:, o + k: o + k + 512],
                            start=(k == 0 and kh == 0), stop=(k == 4 and kh == 1))
                nc.vector.tensor_scalar(
                    out=relu_sb[:, o:o + 512], in0=cps[:],
                    scalar1=c1b[:, 0:1], scalar2=0.0, op0=Alu.add, op1=Alu.max)
                rf = relu_sb[:]
                ev = bass.AP(tensor=rf.tensor, offset=rf.offset + o,
                             ap=[[rf.ap[0][0], 128], [2, 256]])
                od = bass.AP(tensor=rf.tensor, offset=rf.offset + o + 1,
                             ap=[[rf.ap[0][0], 128], [2, 256]])
                nc.vector.tensor_max(out=x_pool[:, 3 + o // 2: 3 + o // 2 + 256],
                                     in0=ev, in1=od)

            wps = wrm.tile([128, 512], FP, name="warm_ps")
            for c in range(16):
                xg = gp.tile([128, EMB], BF, tag="xg")
                nc.gpsimd.indirect_dma_start(
                    out=xg[:], out_offset=None, in_=d_emb[:],
                    in_offset=bass.IndirectOffsetOnAxis(ap=ids_sb[:, c:c + 1],
                                                        axis=0))
                for h in range(2):
                    pt = gtp.tile([128, 128], BF, tag="pt")
                    nc.tensor.transpose(out=pt[:], in_=xg[:, 128 * h:128 * (h + 1)],
                                        identity=ident[:])
                    nc.vector.tensor_copy(
                        out=x_emb[h][:, 2 + 128 * c:2 + 128 * (c + 1)], in_=pt[:])
                if 2 <= c < 10:
                    # PE pstate warmup while the gather streams in
                    nc.tensor.matmul(out=wps[:], lhsT=c1w[:, 0:128],
                                     rhs=c1w[:, 0:512], start=True, stop=True)
                # issue conv group g once chunks 0..4g+4 are transposed
                if c >= 4 and c % 4 == 0:
                    conv_group(c // 4 - 1)
            conv_group(3)

            # ====== PHASE 3: in_proj (+ folded depthwise conv) + silu =======
            for o in (0, 512):
                zps = []
                for h in range(2):
                    xcp = fe.tile([128, 512], FP, tag="ip")
                    for k in range(4):
                        nc.tensor.matmul(
                            out=xcp[:],
                            lhsT=xcw[:, (k * 2 + h) * 128:(k * 2 + h + 1) * 128],
                            rhs=x_pool[:, o + k: o + k + 512],
                            start=(k == 0), stop=(k == 3))
                    zp = fe.tile([128, 512], FP, tag="ip")
                    nc.tensor.matmul(
                        out=zp[:], lhsT=zw[:, h * 128:(h + 1) * 128],
                        rhs=x_pool[:, 3 + o: 3 + o + 512], start=True, stop=True)
                    zps.append(zp)
                    # xs silu first: it gates x_proj -> dt -> scan start
                    nc.scalar.activation(out=xs_big[:, h * L + o: h * L + o + 512],
                                         in_=xcp[:], func=Act.Silu,
                                         bias=cdb[:, h:h + 1], scale=1.0)

                # ============ PHASE 4: x_proj -> (dt_in, B, C) ==============
                xdp = fe.tile([40, 512], FP, tag="ip")
                for kh in range(2):
                    nc.tensor.matmul(
                        out=xdp[:], lhsT=xpw[:, kh * 40:(kh + 1) * 40],
                        rhs=xs_big[:, kh * L + o: kh * L + o + 512],
                        start=(kh == 0), stop=(kh == 1))
                nc.vector.tensor_copy(out=xdbl_sb[:, o:o + 512], in_=xdp[0:40, :])
                for h in range(2):
                    nc.scalar.activation(out=sz_big[:, h * L + o: h * L + o + 512],
                                         in_=zps[h][:], func=Act.Silu, scale=1.0)

            # ========== PHASE 5: replicate B, C across partitions ===========
            # Two-stage tree: single-row broadcast DMAs bottleneck on the one
            # source partition's read port (~5us per row). Stage 1 makes 8
            # copies of each row spread across 128 partitions (reads spread
            # over 16 source partitions); stage 2 reads 8 partitions 16x each.
            xf = xdbl_sb[:]
            xstep = xf.ap[0][0]

            def stage1(dst, row0):
                src = bass.AP(tensor=xf.tensor, offset=xf.offset + row0 * xstep,
                              ap=[[xstep, 16], [0, 8], [1, L]])
                nc.sync.dma_start(out=dst[:], in_=src)

            btmp = W.tile([128, L], BF, name="btmp")
            ctmp = W.tile([128, L], BF, name="ctmp")
            stage1(btmp, 8)
            stage1(ctmp, 24)

            def stage2(dst_pool, tmp, n):
                tf = tmp[:]
                tstep = tf.ap[0][0]
                src = bass.AP(tensor=tf.tensor,
                              offset=tf.offset + (n * 8) * tstep,
                              ap=[[tstep, 8], [0, 16], [1, L]])
                nc.sync.dma_start(out=dst_pool[:, n * L:(n + 1) * L], in_=src)

            def stage2_gp(dst_pool, tmp, n):
                tf = tmp[:]
                tstep = tf.ap[0][0]
                src = bass.AP(tensor=tf.tensor,
                              offset=tf.offset + (n * 8) * tstep,
                              ap=[[tstep, 8], [0, 16], [1, L]])
                nc.gpsimd.dma_start(out=dst_pool[:, n * L:(n + 1) * L], in_=src)

            for n in range(DS):
                stage2_gp(brep, btmp, n)   # SWDGE queue (Pool idle here)
                stage2(crep, ctmp, n)      # SP queue

            # ================= PHASE 6: dt (softplus) + u = dt*xs ===========
            for h in range(2):
                for o in (0, 512):
                    dtp = fe.tile([128, 512], FP, tag="ip")
                    nc.tensor.matmul(
                        out=dtp[:], lhsT=dtw[0:8, h * 128:(h + 1) * 128],
                        rhs=xdbl_sb[0:8, o:o + 512], start=True, stop=True)
                    nc.scalar.activation(
                        out=dt_big[:, h * L + o: h * L + o + 512], in_=dtp[:],
                        func=Act.Exp, bias=dtb[:, h:h + 1], scale=1.0)
            for c0 in (0, 1024):
                nc.scalar.activation(out=dt_big[:, c0:c0 + 1024],
                                     in_=dt_big[:, c0:c0 + 1024],
                                     func=Act.Ln, bias=1.0, scale=1.0)
            for c0 in (0, 1024):
                nc.vector.tensor_mul(out=u_big[:, c0:c0 + 1024],
                                     in0=dt_big[:, c0:c0 + 1024],
                                     in1=xs_big[:, c0:c0 + 1024])


        # ================= PHASE 7: selective scan ==========================
        with tc.tile_pool(name="yp", bufs=1, space="PSUM") as ypp:
            yp = [ypp.tile([128, L], FP, name=f"yp{h}") for h in range(2)]

            def issue_exp(n):
                # dA = exp(A_n * dt); boundary col L stays 0 (resets h at the
                # half-1 start); half-1 cols shifted by one (t=0 dA unused).
                dA = dA_r[n % NBA]
                nc.scalar.activation(out=dA[:, 0:L], in_=dt_big[:, 0:L],
                                     func=Act.Exp, scale=asc[:, n:n + 1])
                nc.scalar.activation(out=dA[:, L + 1:2 * L],
                                     in_=dt_big[:, L + 1:2 * L],
                                     func=Act.Exp, scale=asc[:, DS + n:DS + n + 1])

            for i in range(4):
                issue_exp(i)
            for n in range(DS):
                dA, dBu, ht, hC = (dA_r[n % NBA], dBu_r[n % NB],
                                   ht_r[n % NB], hC_r[n % NB])
                nc.vector.tensor_mul(out=dBu[:], in0=u_big[:],
                                     in1=_rep2(brep, n * L, L))
                if n + 4 < DS:
                    issue_exp(n + 4)
                nc.vector.tensor_tensor_scan(
                    out=ht[:], data0=dA[:], data1=dBu[:],
                    initial=0.0, op0=Alu.mult, op1=Alu.add)
                nc.vector.tensor_mul(out=hC[:], in0=ht[:],
                                     in1=_rep2(crep, n * L, L))
                for h in range(2):
                    for oo in (0, 512):
                        nc.tensor.matmul(
                            out=yp[h][:, oo:oo + 512], lhsT=ident[:],
                            rhs=hC[:, h * L + oo: h * L + oo + 512],
                            start=(n == 0), stop=(n == DS - 1))
                if n == 0:
                    # xs*D contribution, accumulated once into yp
                    for h in range(2):
                        for oo in (0, 512):
                            nc.tensor.matmul(
                                out=yp[h][:, oo:oo + 512],
                                lhsT=ddw[:, h * 128:(h + 1) * 128],
                                rhs=xs_big[:, h * L + oo: h * L + oo + 512],
                                start=False, stop=False)

            # ============= PHASE 8: gate, mean, out_proj, fc ================
            ysum = W.tile([128, 2], FP, name="ysum")
            y2 = [W.tile([128, L], BF, name=f"y2_{h}") for h in range(2)]
            for h in range(2):
                nc.vector.scalar_tensor_tensor(
                    out=y2[h][:], in0=sz_big[:, h * L:(h + 1) * L],
                    scalar=1.0, in1=yp[h][:],
                    op0=Alu.mult, op1=Alu.mult,
                    accum_out=ysum[:, h:h + 1])

            yop = ypp.tile([128, 1], FP, name="yop")
            for h in range(2):
                nc.tensor.matmul(out=yop[:], lhsT=opw[:, h * 128:(h + 1) * 128],
                                 rhs=ysum[:, h:h + 1], start=(h == 0),
                                 stop=(h == 1))
            ymean = W.tile([128, 1], FP, name="ymean")
            nc.vector.tensor_copy(out=ymean[:], in_=yop[:])
            fcp = ypp.tile([10, 1], FP, name="fcp")
            nc.tensor.matmul(out=fcp[:], lhsT=fcw[:, 0:NCLS], rhs=ymean[:],
                             start=True, stop=True)
            out_sb = W.tile([10, 1], FP, name="out_sb")
            nc.vector.tensor_scalar_add(out=out_sb[:], in0=fcp[:],
                                        scalar1=fcb[0:10, 0:1])
        out_dst = bass.AP(tensor=d_out[:].tensor, offset=0, ap=[[1, NCLS]])
        out_src = bass.AP(tensor=out_sb[:].tensor, offset=out_sb[:].offset,
                          ap=[[out_sb[:].ap[0][0], NCLS]])
        nc.sync.dma_start(out=out_dst, in_=out_src)

    import concourse.bacc as _bacc
    _orig_gat = _bacc.get_activation_tables

    def _gat(arch):
        t = dict(_orig_gat(arch))
        if "natural_log_exp_and_others" in t:
            t2 = {"natural_log_exp_and_others": t.pop("natural_log_exp_and_others")}
            t2.update(t)
            return t2
        return t

    _bacc.get_activation_tables = _gat
    try:
        nc.compile()
    finally:
        _bacc.get_activation_tables = _orig_gat
    return nc


def prep_consts(inputs):
    """Host-side weight transforms (parameters only)."""
    f32 = np.float32
    bf16 = ml_dtypes.bfloat16
    emb = np.ascontiguousarray(np.asarray(inputs["emb"], f32).astype(bf16))
    conv1_w = np.asarray(inputs["conv1_w"], f32)      # (128, 256, 5)
    conv1_b = np.asarray(inputs["conv1_b"], f32)
    in_proj_w = np.asarray(inputs["in_proj_w"], f32)  # (512, 128)
    convd_w = np.asarray(inputs["convd_w"], f32)      # (256, 1, 4)
    convd_b = np.asarray(inputs["convd_b"], f32)
    x_proj_w = np.asarray(inputs["x_proj_w"], f32)    # (40, 256)
    dt_proj_w = np.asarray(inputs["dt_proj_w"], f32)  # (256, 8)
    dt_proj_b = np.asarray(inputs["dt_proj_b"], f32)
    A_log = np.asarray(inputs["A_log"], f32)          # (256, 16)
    Dv = np.asarray(inputs["D"], f32)
    out_proj_w = np.asarray(inputs["out_proj_w"], f32)  # (128, 256)
    fc_w = np.asarray(inputs["fc_w"], f32)            # (10, 128)
    fc_b = np.asarray(inputs["fc_b"], f32)

    c1w = np.zeros((128, 5, 2, 128), f32)
    for k in range(5):
        for kh in range(2):
            c1w[:, k, kh, :] = conv1_w[:, kh * 128:(kh + 1) * 128, k].T
    c1w = c1w.reshape(128, -1)

    Wx = in_proj_w[:DI]          # (256, 128)
    xcw = np.zeros((128, 4, 2, 128), f32)
    for k in range(4):
        Wxk = convd_w[:, 0, k][:, None] * Wx          # (256, 128)
        for mc in range(2):
            xcw[:, k, mc, :] = Wxk[mc * 128:(mc + 1) * 128, :].T
    xcw = xcw.reshape(128, -1)

    Wz = in_proj_w[DI:]
    zw = np.zeros((128, 2, 128), f32)
    for mc in range(2):
        zw[:, mc, :] = Wz[mc * 128:(mc + 1) * 128, :].T
    zw = zw.reshape(128, -1)

    xpw = np.zeros((128, 2, 40), f32)
    for kh in range(2):
        xpw[:, kh, :] = x_proj_w[:, kh * 128:(kh + 1) * 128].T
    xpw = xpw.reshape(128, -1)

    dtw = np.zeros((8, 2, 128), f32)
    for mc in range(2):
        dtw[:, mc, :] = dt_proj_w[mc * 128:(mc + 1) * 128, :].T
    dtw = dtw.reshape(8, -1)

    A = -np.exp(A_log)           # (256, 16)
    asc = np.zeros((128, 2 * DS), f32)
    asc[:, :DS] = A[:128]        # half-0 channels
    asc[:, DS:] = A[128:]        # half-1 channels

    opw = np.zeros((128, 2, 128), f32)
    for kh in range(2):
        opw[:, kh, :] = out_proj_w[:, kh * 128:(kh + 1) * 128].T
    opw = opw.reshape(128, -1)

    fcw = (fc_w / float(L)).T.copy()                  # (128, 10)

    ddw = np.zeros((128, 2, 128), f32)
    for hh in range(2):
        ddw[:, hh, :] = np.diag(Dv[hh * 128:(hh + 1) * 128])
    ddw = ddw.reshape(128, -1)

    consts = {
        "emb": emb, "ddw": ddw.astype(bf16),
        "c1w": c1w.astype(bf16), "xcw": xcw.astype(bf16), "zw": zw.astype(bf16),
        "xpw": xpw.astype(bf16), "dtw": dtw.astype(bf16),
        "asc": asc, "opw": opw, "fcw": fcw,
        "ident": np.eye(128, dtype=f32).astype(bf16),
        "c1b": conv1_b.reshape(128, 1).copy(),
        "cdb": convd_b.reshape(2, 128).T.copy(),
        "dtb": dt_proj_b.reshape(2, 128).T.copy(),
        "dvec": Dv.reshape(2, 128).T.copy(),
        "fcb": fc_b.reshape(10, 1).copy(),
    }
    return consts


_CACHE = {}


def kernel(**inputs) -> np.ndarray:
    ids = np.asarray(inputs["ids"])
    assert ids.shape == (8, SEQ), ids.shape
    ids32 = np.ascontiguousarray(ids, dtype=np.int32)

    if "nc" not in _CACHE:
        _CACHE["nc"] = build_program()
    nc = _CACHE["nc"]
    nonce_name = [t for t in (a.memorylocations[0].name
                              for a in nc.m.functions[0].allocations
                              if getattr(a, "kind", None) == "ExternalInput"
                              and a.memorylocations)
                  if t.startswith("nonce_")][0]

    consts = prep_consts(inputs)
    in_maps = []
    for b in range(8):
        m = dict(consts)
        m["ids"] = np.ascontiguousarray(ids32[b].reshape(16, 128).T)
        m[nonce_name] = np.zeros((1, 1), np.float32)
        in_maps.append(m)

    trace = os.environ.get("MAMBA_TRACE", "0") == "1"
    res = run_bass_kernel_spmd(nc, in_maps, core_ids=list(range(8)), trace=trace)
    _CACHE["last_results"] = res
    out = np.stack([res.results[b]["out"] for b in range(8)]).astype(np.float32)
    return out


# revision 25
# speedup vs baseline: 1.3534x; 1.0010x over previous
"""Trainium2 Bass kernel for CNN+Mamba classifier.

Contract: kernel(**inputs) takes FULL unsharded inputs (numpy), returns FULL
(8, 10) float32 output. Internally shards data-parallel over batch across 8
NeuronCores (1 example per core), with all parameters replicated.

v3 architecture:
  - Embedding gather via gpsimd indirect DMA + PE transposes (proven path).
  - Selective scan tiled channel-major: one [128, 2048] tile per state n
    (both 128-channel halves merged along the free dim; dA boundary column
    kept 0 so the recurrence resets between halves). No replication matmuls,
    no PSUM->SBUF copies.
  - B/C rows replicated across partitions by DMA; read twice via 0-stride AP.
  - All scan-phase elementwise work on DVE (GpSimd shares SBUF ports with
    DVE, so offloading there is a wash); exps on Act; y accumulated over
    states via identity-lhsT PSUM matmuls; mean folded into stt accum_out.

Self-contained: hardcodes all shapes; no sibling imports.
"""

import os
from contextlib import ExitStack

import numpy as np
import ml_dtypes

import concourse.bass as bass
import concourse.bacc as bacc
import concourse.tile as tile
from concourse import mybir
from concourse.bass_utils import run_bass_kernel_spmd

FP = mybir.dt.float32
BF = mybir.dt.bfloat16
I32 = mybir.dt.int32

VOCAB, EMB, NCLS, SEQ = 50000, 256, 10, 2048
DM, DI, DS, DCONV, DTR = 128, 256, 16, 4, 8
L = SEQ // 2  # 1024 after maxpool


def _rep2(t_ap, col0, n):
    """in1 AP reading cols [col0, col0+n) of a [128, *] tile twice (0-stride)."""
    full = t_ap[:]
    return bass.AP(tensor=full.tensor, offset=full.offset + col0,
                   ap=[[full.ap[0][0], 128], [0, 2], [1, n]])


def _row_bcast(t_ap, row, n):
    """DMA source AP: row `row` of tile, broadcast to 128 partitions."""
    full = t_ap[:]
    pstep = full.ap[0][0]
    return bass.AP(tensor=full.tensor, offset=full.offset + row * pstep,
                   ap=[[pstep, 1], [0, 128], [1, n]])


def build_program():
    nc = bacc.Bacc("TRN2", target_bir_lowering=False, debug=False, num_devices=8)

    d_ids = nc.dram_tensor("ids", [128, 16], I32, kind="ExternalInput")
    d_emb = nc.dram_tensor("emb", [VOCAB, EMB], BF, kind="ExternalInput")
    d_c1w = nc.dram_tensor("c1w", [128, 5 * 2 * 128], BF, kind="ExternalInput")
    d_xcw = nc.dram_tensor("xcw", [128, 4 * 2 * 128], BF, kind="ExternalInput")
    d_zw = nc.dram_tensor("zw", [128, 2 * 128], BF, kind="ExternalInput")
    d_xpw = nc.dram_tensor("xpw", [128, 2 * 40], BF, kind="ExternalInput")
    d_dtw = nc.dram_tensor("dtw", [8, 2 * 128], BF, kind="ExternalInput")
    d_asc = nc.dram_tensor("asc", [128, 2 * DS], FP, kind="ExternalInput")
    d_opw = nc.dram_tensor("opw", [128, 2 * 128], FP, kind="ExternalInput")
    d_fcw = nc.dram_tensor("fcw", [128, NCLS], FP, kind="ExternalInput")
    d_ident = nc.dram_tensor("ident", [128, 128], BF, kind="ExternalInput")
    d_c1b = nc.dram_tensor("c1b", [128, 1], FP, kind="ExternalInput")
    d_cdb = nc.dram_tensor("cdb", [128, 2], FP, kind="ExternalInput")
    d_dtb = nc.dram_tensor("dtb", [128, 2], FP, kind="ExternalInput")
    d_dvec = nc.dram_tensor("dvec", [128, 2], FP, kind="ExternalInput")
    d_ddw = nc.dram_tensor("ddw", [128, 2 * 128], BF, kind="ExternalInput")
    d_fcb = nc.dram_tensor("fcb", [10, 1], FP, kind="ExternalInput")

    import uuid
    nonce = uuid.uuid4().hex[:12]
    d_nonce = nc.dram_tensor(f"nonce_{nonce}", [1, 1], FP, kind="ExternalInput")
    d_out = nc.dram_tensor("out", [NCLS], FP, kind="ExternalOutput")

    Alu = mybir.AluOpType
    Act = mybir.ActivationFunctionType

    with ExitStack() as ctx:
        tc = ctx.enter_context(tile.TileContext(nc))
        W = ctx.enter_context(tc.tile_pool(name="w", bufs=1))
        ids_sb = W.tile([128, 16], I32, name="ids_sb")
        nc.sync.dma_start(out=ids_sb[:], in_=d_ids[:])
        nonce_sb = W.tile([1, 1], FP, name="nonce_sb")
        nc.sync.dma_start(out=nonce_sb[:], in_=d_nonce[:])
        pwarm = W.tile([128, 2], FP, name="pool_warm")
        nc.gpsimd.memset(pwarm[:], 0.0)

        def load(dram, shape, dtype=FP):
            t = W.tile(list(shape), dtype, name=f"w_{dram.name}")
            nc.sync.dma_start(out=t[:], in_=dram[:])
            return t
        c1w = load(d_c1w, (128, 5 * 2 * 128), BF)
        xcw = load(d_xcw, (128, 4 * 2 * 128), BF)
        zw = load(d_zw, (128, 2 * 128), BF)
        xpw = load(d_xpw, (128, 2 * 40), BF)
        dtw = load(d_dtw, (8, 2 * 128), BF)
        asc = load(d_asc, (128, 2 * DS))
        opw = load(d_opw, (128, 2 * 128))
        fcw = load(d_fcw, (128, NCLS))
        ident = load(d_ident, (128, 128), BF)
        c1b = load(d_c1b, (128, 1))
        cdb = load(d_cdb, (128, 2))
        dtb = load(d_dtb, (128, 2))
        dvec = load(d_dvec, (128, 2))
        ddw = load(d_ddw, (128, 2 * 128), BF)
        fcb = load(d_fcb, (10, 1))

        # ---- persistent intermediates ----
        x_emb = [W.tile([128, SEQ + 4], BF, name=f"x_emb{_}") for _ in range(2)]
        for h in range(2):
            nc.vector.memset(x_emb[h][:, 0:2], 0.0)
            nc.vector.memset(x_emb[h][:, SEQ + 2:SEQ + 4], 0.0)
        relu_sb = W.tile([128, SEQ], BF, name="relu_sb")
        x_pool = W.tile([128, L + 3], BF, name="x_pool")
        nc.vector.memset(x_pool[:, 0:3], 0.0)
        xs_big = W.tile([128, 2 * L], BF, name="xs_big")
        sz_big = W.tile([128, 2 * L], BF, name="sz_big")
        dt_big = W.tile([128, 2 * L], FP, name="dt_big")
        u_big = W.tile([128, 2 * L], BF, name="u_big")
        xdbl_sb = W.tile([40, L], BF, name="xdbl_sb")
        brep = W.tile([128, DS * L], BF, name="brep")
        crep = W.tile([128, DS * L], BF, name="crep")

        # scan working rings; dA col L is the recurrence reset (stays 0)
        NB = 4
        NBA = 6
        dA_r = [W.tile([128, 2 * L], BF, name=f"dA{i}") for i in range(NBA)]
        for i in range(NBA):
            nc.vector.memset(dA_r[i][:, L:L + 1], 0.0)
        dBu_r = [W.tile([128, 2 * L], BF, name=f"dBu{i}") for i in range(NB)]
        ht_r = [W.tile([128, 2 * L], BF, name=f"ht{i}") for i in range(NB)]
        hC_r = [W.tile([128, 2 * L], BF, name=f"hC{i}") for i in range(NB)]

        # preload the silu ACT table (first real Act op is silu)
        scratch = W.tile([128, 2], FP, name="act_scratch")
        nc.vector.memset(scratch[:], 1.0)
        nc.scalar.activation(out=scratch[:, 0:1], in_=scratch[:, 0:1],
                             func=Act.Silu, scale=1.0)

        # ============ PHASE 1+2: gather + transpose | conv + relu + max =====
        with tc.tile_pool(name="g", bufs=8) as gp, \
             tc.tile_pool(name="gt", bufs=2, space="PSUM") as gtp, \
             tc.tile_pool(name="wrm", bufs=1, space="PSUM") as wrm, \
             tc.tile_pool(name="cps", bufs=2, space="PSUM") as cpp, \
             tc.tile_pool(name="fe", bufs=3, space="PSUM") as fe:

            def conv_group(g):
                o = 512 * g
                cps = cpp.tile([128, 512], FP, tag="cps")
                for k in range(5):
                    for kh in range(2):
                        nc.tensor.matmul(
                            out=cps[:],
                            lhsT=c1w[:, (k * 2 + kh) * 128:(k * 2 + kh + 1) * 128],
                            rhs=x_emb[kh][:, o + k: o + k + 512],
                            start=(k == 0 and kh == 0), stop=(k == 4 and kh == 1))
                nc.vector.tensor_scalar(
                    out=relu_sb[:, o:o + 512], in0=cps[:],
                    scalar1=c1b[:, 0:1], scalar2=0.0, op0=Alu.add, op1=Alu.max)
                rf = relu_sb[:]
                ev = bass.AP(tensor=rf.tensor, offset=rf.offset + o,
                             ap=[[rf.ap[0][0], 128], [2, 256]])
                od = bass.AP(tensor=rf.tensor, offset=rf.offset + o + 1,
                             ap=[[rf.ap[0][0], 128], [2, 256]])
                nc.vector.tensor_max(out=x_pool[:, 3 + o // 2: 3 + o // 2 + 256],
                                     in0=ev, in1=od)

            wps = wrm.tile([128, 512], FP, name="warm_ps")
            for c in range(16):
                xg = gp.tile([128, EMB], BF, tag="xg")
                nc.gpsimd.indirect_dma_start(
                    out=xg[:], out_offset=None, in_=d_emb[:],
                    in_offset=bass.IndirectOffsetOnAxis(ap=ids_sb[:, c:c + 1],
                                                        axis=0))
                for h in range(2):
                    pt = gtp.tile([128, 128], BF, tag="pt")
                    nc.tensor.transpose(out=pt[:], in_=xg[:, 128 * h:128 * (h + 1)],
                                        identity=ident[:])
                    nc.vector.tensor_copy(
                        out=x_emb[h][:, 2 + 128 * c:2 + 128 * (c + 1)], in_=pt[:])
                if 2 <= c < 10:
                    # PE pstate warmup while the gather streams in
                    nc.tensor.matmul(out=wps[:], lhsT=c1w[:, 0:128],
                                     rhs=c1w[:, 0:512], start=True, stop=True)
                # issue conv group g once chunks 0..4g+4 are transposed
                if c >= 4 and c % 4 == 0:
                    conv_group(c // 4 - 1)
            conv_group(3)

            # ====== PHASE 3: in_proj (+ folded depthwise conv) + silu =======
            for o in (0, 512):
                zps = []
                for h in range(2):
                    xcp = fe.tile([128, 512], FP, tag="ip")
                    for k in range(4):
                        nc.tensor.matmul(
                            out=xcp[:],
                            lhsT=xcw[:, (k * 2 + h) * 128:(k * 2 + h + 1) * 128],
                            rhs=x_pool[:, o + k: o + k + 512],
                            start=(k == 0), stop=(k == 3))
                    zp = fe.tile([128, 512], FP, tag="ip")
                    nc.tensor.matmul(
                        out=zp[:], lhsT=zw[:, h * 128:(h + 1) * 128],
                        rhs=x_pool[:, 3 + o: 3 + o + 512], start=True, stop=True)
                    zps.append(zp)
                    # xs silu first: it gates x_proj -> dt -> scan start
                    nc.scalar.activation(out=xs_big[:, h * L + o: h * L + o + 512],
                                         in_=xcp[:], func=Act.Silu,
                                         bias=cdb[:, h:h + 1], scale=1.0)

                # ============ PHASE 4: x_proj -> (dt_in, B, C) ==============
                xdp = fe.tile([40, 512], FP, tag="ip")
                for kh in range(2):
                    nc.tensor.matmul(
                        out=xdp[:], lhsT=xpw[:, kh * 40:(kh + 1) * 40],
                        rhs=xs_big[:, kh * L + o: kh * L + o + 512],
                        start=(kh == 0), stop=(kh == 1))
                nc.vector.tensor_copy(out=xdbl_sb[:, o:o + 512], in_=xdp[0:40, :])
                for h in range(2):
                    nc.scalar.activation(out=sz_big[:, h * L + o: h * L + o + 512],
                                         in_=zps[h][:], func=Act.Silu, scale=1.0)

            # ========== PHASE 5: replicate B, C across partitions ===========
            # Two-stage tree: single-row broadcast DMAs bottleneck on the one
            # source partition's read port (~5us per row). Stage 1 makes 8
            # copies of each row spread across 128 partitions (reads spread
            # over 16 source partitions); stage 2 reads 8 partitions 16x each.
            xf = xdbl_sb[:]
            xstep = xf.ap[0][0]

            def stage1(dst, row0):
                src = bass.AP(tensor=xf.tensor, offset=xf.offset + row0 * xstep,
                              ap=[[xstep, 16], [0, 8], [1, L]])
                nc.sync.dma_start(out=dst[:], in_=src)

            btmp = W.tile([128, L], BF, name="btmp")
            ctmp = W.tile([128, L], BF, name="ctmp")
            stage1(btmp, 8)
            stage1(ctmp, 24)

            def stage2(dst_pool, tmp, n):
                tf = tmp[:]
                tstep = tf.ap[0][0]
                src = bass.AP(tensor=tf.tensor,
                              offset=tf.offset + (n * 8) * tstep,
                              ap=[[tstep, 8], [0, 16], [1, L]])
                nc.sync.dma_start(out=dst_pool[:, n * L:(n + 1) * L], in_=src)

            def stage2_gp(dst_pool, tmp, n):
                tf = tmp[:]
                tstep = tf.ap[0][0]
                src = bass.AP(tensor=tf.tensor,
                              offset=tf.offset + (n * 8) * tstep,
                              ap=[[tstep, 8], [0, 16], [1, L]])
                nc.gpsimd.dma_start(out=dst_pool[:, n * L:(n + 1) * L], in_=src)

            for n in range(DS):
                stage2_gp(brep, btmp, n)   # SWDGE queue (Pool idle here)
                stage2(crep, ctmp, n)      # SP queue

            # ================= PHASE 6: dt (softplus) + u = dt*xs ===========
            for h in range(2):
                for o in (0, 512):
                    dtp = fe.tile([128, 512], FP, tag="ip")
                    nc.tensor.matmul(
                        out=dtp[:], lhsT=dtw[0:8, h * 128:(h + 1) * 128],
                        rhs=xdbl_sb[0:8, o:o + 512], start=True, stop=True)
                    nc.scalar.activation(
                        out=dt_big[:, h * L + o: h * L + o + 512], in_=dtp[:],
                        func=Act.Exp, bias=dtb[:, h:h + 1], scale=1.0)
            for c0 in (0, 1024):
                nc.scalar.activation(out=dt_big[:, c0:c0 + 1024],
                                     in_=dt_big[:, c0:c0 + 1024],
                                     func=Act.Ln, bias=1.0, scale=1.0)
            for c0 in (0, 1024):
                nc.vector.tensor_mul(out=u_big[:, c0:c0 + 1024],
                                     in0=dt_big[:, c0:c0 + 1024],
                                     in1=xs_big[:, c0:c0 + 1024])


        # ================= PHASE 7: selective scan ==========================
        with tc.tile_pool(name="yp", bufs=1, space="PSUM") as ypp:
            yp = [ypp.tile([128, L], FP, name=f"yp{h}") for h in range(2)]

            def issue_exp(n):
                # dA = exp(A_n * dt); boundary col L stays 0 (resets h at the
                # half-1 start); half-1 cols shifted by one (t=0 dA unused).
                dA = dA_r[n % NBA]
                nc.scalar.activation(out=dA[:, 0:L], in_=dt_big[:, 0:L],
                                     func=Act.Exp, scale=asc[:, n:n + 1])
                nc.scalar.activation(out=dA[:, L + 1:2 * L],
                                     in_=dt_big[:, L + 1:2 * L],
                                     func=Act.Exp, scale=asc[:, DS + n:DS + n + 1])

            for i in range(5):
                issue_exp(i)
            for n in range(DS):
                dA, dBu, ht, hC = (dA_r[n % NBA], dBu_r[n % NB],
                                   ht_r[n % NB], hC_r[n % NB])
                nc.vector.tensor_mul(out=dBu[:], in0=u_big[:],
                                     in1=_rep2(brep, n * L, L))
                if n + 5 < DS:
                    issue_exp(n + 5)
                nc.vector.tensor_tensor_scan(
                    out=ht[:], data0=dA[:], data1=dBu[:],
                    initial=0.0, op0=Alu.mult, op1=Alu.add)
                nc.vector.tensor_mul(out=hC[:], in0=ht[:],
                                     in1=_rep2(crep, n * L, L))
                for h in range(2):
                    for oo in (0, 512):
                        nc.tensor.matmul(
                            out=yp[h][:, oo:oo + 512], lhsT=ident[:],
                            rhs=hC[:, h * L + oo: h * L + oo + 512],
                            start=(n == 0), stop=(n == DS - 1))
                if n == 0:
                    # xs*D contribution, accumulated once into yp
                    for h in range(2):
                        for oo in (0, 512):
                            nc.tensor.matmul(
                                out=yp[h][:, oo:oo + 512],
                                lhsT=ddw[:, h * 128:(h + 1) * 128],
                                rhs=xs_big[:, h * L + oo: h * L + oo + 512],
                                start=False, stop=False)

            # ============= PHASE 8: gate, mean, out_proj, fc ================
            ysum = W.tile([128, 2], FP, name="ysum")
            y2 = [W.tile([128, L], BF, name=f"y2_{h}") for h in range(2)]
            for h in range(2):
                nc.vector.scalar_tensor_tensor(
                    out=y2[h][:], in0=sz_big[:, h * L:(h + 1) * L],
                    scalar=1.0, in1=yp[h][:],
                    op0=Alu.mult, op1=Alu.mult,
                    accum_out=ysum[:, h:h + 1])

            yop = ypp.tile([128, 1], FP, name="yop")
            for h in range(2):
                nc.tensor.matmul(out=yop[:], lhsT=opw[:, h * 128:(h + 1) * 128],
                                 rhs=ysum[:, h:h + 1], start=(h == 0),
                                 stop=(h == 1))
            ymean = W.tile([128, 1], FP, name="ymean")
            nc.vector.tensor_copy(out=ymean[:], in_=yop[:])
            fcp = ypp.tile([10, 1], FP, name="fcp")
            nc.tensor.matmul(out=fcp[:], lhsT=fcw[:, 0:NCLS], rhs=ymean[:],
                             start=True, stop=True)
            out_sb = W.tile([10, 1], FP, name="out_sb")
            nc.vector.tensor_scalar_add(out=out_sb[:], in0=fcp[:],
                                        scalar1=fcb[0:10, 0:1])
        out_dst = bass.AP(tensor=d_out[:].tensor, offset=0, ap=[[1, NCLS]])
        out_src = bass.AP(tensor=out_sb[:].tensor, offset=out_sb[:].offset,
                          ap=[[out_sb[:].ap[0][0], NCLS]])
        nc.sync.dma_start(out=out_dst, in_=out_src)

    import concourse.bacc as _bacc
    _orig_gat = _bacc.get_activation_tables

    def _gat(arch):
        t = dict(_orig_gat(arch))
        if "natural_log_exp_and_others" in t:
            t2 = {"natural_log_exp_and_others": t.pop("natural_log_exp_and_others")}
            t2.update(t)
            return t2
        return t

    _bacc.get_activation_tables = _gat
    try:
        nc.compile()
    finally:
        _bacc.get_activation_tables = _orig_gat
    return nc


def prep_consts(inputs):
    """Host-side weight transforms (parameters only)."""
    f32 = np.float32
    bf16 = ml_dtypes.bfloat16
    emb = np.ascontiguousarray(np.asarray(inputs["emb"], f32).astype(bf16))
    conv1_w = np.asarray(inputs["conv1_w"], f32)      # (128, 256, 5)
    conv1_b = np.asarray(inputs["conv1_b"], f32)
    in_proj_w = np.asarray(inputs["in_proj_w"], f32)  # (512, 128)
    convd_w = np.asarray(inputs["convd_w"], f32)      # (256, 1, 4)
    convd_b = np.asarray(inputs["convd_b"], f32)
    x_proj_w = np.asarray(inputs["x_proj_w"], f32)    # (40, 256)
    dt_proj_w = np.asarray(inputs["dt_proj_w"], f32)  # (256, 8)
    dt_proj_b = np.asarray(inputs["dt_proj_b"], f32)
    A_log = np.asarray(inputs["A_log"], f32)          # (256, 16)
    Dv = np.asarray(inputs["D"], f32)
    out_proj_w = np.asarray(inputs["out_proj_w"], f32)  # (128, 256)
    fc_w = np.asarray(inputs["fc_w"], f32)            # (10, 128)
    fc_b = np.asarray(inputs["fc_b"], f32)

    c1w = np.zeros((128, 5, 2, 128), f32)
    for k in range(5):
        for kh in range(2):
            c1w[:, k, kh, :] = conv1_w[:, kh * 128:(kh + 1) * 128, k].T
    c1w = c1w.reshape(128, -1)

    Wx = in_proj_w[:DI]          # (256, 128)
    xcw = np.zeros((128, 4, 2, 128), f32)
    for k in range(4):
        Wxk = convd_w[:, 0, k][:, None] * Wx          # (256, 128)
        for mc in range(2):
            xcw[:, k, mc, :] = Wxk[mc * 128:(mc + 1) * 128, :].T
    xcw = xcw.reshape(128, -1)

    Wz = in_proj_w[DI:]
    zw = np.zeros((128, 2, 128), f32)
    for mc in range(2):
        zw[:, mc, :] = Wz[mc * 128:(mc + 1) * 128, :].T
    zw = zw.reshape(128, -1)

    xpw = np.zeros((128, 2, 40), f32)
    for kh in range(2):
        xpw[:, kh, :] = x_proj_w[:, kh * 128:(kh + 1) * 128].T
    xpw = xpw.reshape(128, -1)

    dtw = np.zeros((8, 2, 128), f32)
    for mc in range(2):
        dtw[:, mc, :] = dt_proj_w[mc * 128:(mc + 1) * 128, :].T
    dtw = dtw.reshape(8, -1)

    A = -np.exp(A_log)           # (256, 16)
    asc = np.zeros((128, 2 * DS), f32)
    asc[:, :DS] = A[:128]        # half-0 channels
    asc[:, DS:] = A[128:]        # half-1 channels

    opw = np.zeros((128, 2, 128), f32)
    for kh in range(2):
        opw[:, kh, :] = out_proj_w[:, kh * 128:(kh + 1) * 128].T
    opw = opw.reshape(128, -1)

    fcw = (fc_w / float(L)).T.copy()                  # (128, 10)

    ddw = np.zeros((128, 2, 128), f32)
    for hh in range(2):
        ddw[:, hh, :] = np.diag(Dv[hh * 128:(hh + 1) * 128])
    ddw = ddw.reshape(128, -1)

    consts = {
        "emb": emb, "ddw": ddw.astype(bf16),
        "c1w": c1w.astype(bf16), "xcw": xcw.astype(bf16), "zw": zw.astype(bf16),
        "xpw": xpw.astype(bf16), "dtw": dtw.astype(bf16),
        "asc": asc, "opw": opw, "fcw": fcw,
        "ident": np.eye(128, dtype=f32).astype(bf16),
        "c1b": conv1_b.reshape(128, 1).copy(),
        "cdb": convd_b.reshape(2, 128).T.copy(),
        "dtb": dt_proj_b.reshape(2, 128).T.copy(),
        "dvec": Dv.reshape(2, 128).T.copy(),
        "fcb": fc_b.reshape(10, 1).copy(),
    }
    return consts


_CACHE = {}


def kernel(**inputs) -> np.ndarray:
    ids = np.asarray(inputs["ids"])
    assert ids.shape == (8, SEQ), ids.shape
    ids32 = np.ascontiguousarray(ids, dtype=np.int32)

    if "nc" not in _CACHE:
        _CACHE["nc"] = build_program()
    nc = _CACHE["nc"]
    nonce_name = [t for t in (a.memorylocations[0].name
                              for a in nc.m.functions[0].allocations
                              if getattr(a, "kind", None) == "ExternalInput"
                              and a.memorylocations)
                  if t.startswith("nonce_")][0]

    consts = prep_consts(inputs)
    in_maps = []
    for b in range(8):
        m = dict(consts)
        m["ids"] = np.ascontiguousarray(ids32[b].reshape(16, 128).T)
        m[nonce_name] = np.zeros((1, 1), np.float32)
        in_maps.append(m)

    trace = os.environ.get("MAMBA_TRACE", "0") == "1"
    res = run_bass_kernel_spmd(nc, in_maps, core_ids=list(range(8)), trace=trace)
    _CACHE["last_results"] = res
    out = np.stack([res.results[b]["out"] for b in range(8)]).astype(np.float32)
    return out
